# revision 13
# baseline (speedup 1.0000x reference)
"""Causal latent (linear) attention kernel for 8 Trainium2 NeuronCores.

Sharding: core c handles batch b = c//2 and head-group hg = c%2 (8 of 16
heads).  The (B,H,L,L) scan state is independent per (b,h) so there are no
cross-device transfers; each core emits a partial (T, D) output (its 512
y-dims times w_out rows) and the host sums the two partials per batch.

Algorithm (chunked linear attention, chunk C=256):
    q,k,v = x @ w.T  (per-head L=64)
    eq = exp(q/8); kexp = exp(k/8); knorm = cumsum(kexp + 1e-6)
    qs = eq / (Z * knorm),  Z[t] = sum_l eq[t,l]   (per head)
    per chunk: A = kexp_c @ qs_c^T (masked s<=t)
               Y_c = qs_c @ S + A^T-contracted v_c ; S += kexp_c^T v_c
    out = (Y heads concat) @ w_out

All matmuls run in float32r (reduced-precision fp32 PE mode, ~1.6e-4 rel
err measured) which streams at 1 cycle/row for free dims >= 256.  f32r
matmuls cannot target column-offset PSUM partitions, so all M=64 outputs
land on partitions 0-63 and the output projection contracts per head (K=64).
"""

import numpy as np

import concourse.bass as bass
import concourse.tile as tile
from concourse import mybir
from concourse.bass import ds
from concourse.bass_utils import run_bass_kernel_spmd
from concourse.tile import add_dep_helper

F32 = mybir.dt.float32
F32R = mybir.dt.float32r
AF = mybir.ActivationFunctionType
OP = mybir.AluOpType

B, T, D = 4, 2048, 1024
H, L = 16, 64
NP = 4           # head-pairs per core
CQ = 512         # quarter (outer tile) size along T
NQ = T // CQ     # 4
CH = 256         # attention chunk
SCALE = 0.125    # 1/sqrt(L)


def drop_sem_isa(nc):
    """The end-of-kernel semaphore RANGE_CLEAR (InstISA op 176) fails walrus
    codegen ("ISA wrong length") for larger sem ranges in this build.  NRT
    re-initializes semaphore state per execution, so drop it (verified: 3
    repeated executions stay correct).  Its waits move onto a NoOp."""
    n = 0
    for f in nc.m.functions:
        for blk in f.blocks:
            keep = []
            for inst in blk.instructions:
                if type(inst).__name__ == "InstISA":
                    n += 1
                    si = inst.sync_info
                    if si is not None and si.on_wait:
                        nop = mybir.InstNoOp(name=f"{inst.name}-del", ins=[], outs=[])
                        nop.engine = inst.engine
                        nop.sync_info = si
                        keep.append(nop)
                    continue
                keep.append(inst)
            blk.instructions = keep
    return n


def split_excess_waits(nc):
    """This walrus build accepts only ONE sync-wait command per instruction.
    Move excess waits onto same-engine NoOps inserted just before."""
    n = 0
    for f in nc.m.functions:
        for blk in f.blocks:
            new_insts = []
            for inst in blk.instructions:
                si = inst.sync_info
                waits = list(si.on_wait) if si is not None else []
                if len(waits) > 1:
                    for i, wchunk in enumerate(waits[:-1]):
                        nop = mybir.InstNoOp(name=f"{inst.name}-ws{i}", ins=[], outs=[])
                        nop.engine = inst.engine
                        nop.sync_info = mybir.SyncInfo(on_wait=[wchunk], on_update=[])
                        new_insts.append(nop)
                        n += 1
                    inst.sync_info = mybir.SyncInfo(
                        on_wait=waits[-1:], on_update=list(si.on_update)
                    )
                new_insts.append(inst)
            blk.instructions = new_insts
    return n


def build_bass(debug=False):
    nc = bass.Bass(trn_type="TRN2")

    xT = nc.dram_tensor("xt", [D, T], F32R, kind="ExternalInput")        # x[b].T
    wt = nc.dram_tensor("wt", [D, 1536], F32R, kind="ExternalInput")     # [q|k|v] cols
    wo = nc.dram_tensor("wo", [8, 64, D], F32R, kind="ExternalInput")    # per-head rows
    m0d = nc.dram_tensor("m0", [128, 256], F32, kind="ExternalInput")    # causal mask
    zmd = nc.dram_tensor("zm", [128, 32], F32R, kind="ExternalInput")    # Z-sum lhsT
    zbd = nc.dram_tensor("zb", [8, 512], F32R, kind="ExternalInput")     # Z-bcast lhsT
    out = nc.dram_tensor("out", [T, D], F32, kind="ExternalOutput")
    if debug:
        dbg_qs = nc.dram_tensor("dbg_qs", [NQ, 128, NP, CQ], F32, kind="ExternalOutput")
        dbg_kn = nc.dram_tensor("dbg_kn", [NQ, 128, NP, CQ], F32, kind="ExternalOutput")
        dbg_s = nc.dram_tensor("dbg_s", [NQ * 2, 128, NP, 64], F32, kind="ExternalOutput")
        dbg_y = nc.dram_tensor("dbg_y", [NQ, 64, 8, CQ], F32, kind="ExternalOutput")

    xT_r = xT[:, :].rearrange("(o p) t -> p o t", p=128)
    wt_r = wt[:, :].rearrange("(o p) j -> p o j", p=128)
    wo_r = wo[:, :, :].rearrange("h p e -> p h e")

    sweeps = []  # instructions the final clock-sweep nops must cover

    with tile.TileContext(nc) as tc:
        with (
            tc.tile_pool(name="const", bufs=1) as const,
            tc.tile_pool(name="xq", bufs=2) as xqp,
            tc.tile_pool(name="qk", bufs=2) as qkp,
            tc.tile_pool(name="kn", bufs=1) as knp,
            tc.tile_pool(name="natp", bufs=1) as natp,
            tc.tile_pool(name="abar", bufs=2) as abp,
            tc.tile_pool(name="yt", bufs=1) as ytp,
            tc.tile_pool(name="zsb", bufs=1) as zsbp,
            tc.tile_pool(name="ob", bufs=2) as obp,
            tc.tile_pool(name="s_ps", bufs=1, space="PSUM") as s_ps,
            tc.tile_pool(name="mm_ps", bufs=2, space="PSUM") as mm_ps,
            tc.tile_pool(name="a_ps", bufs=1, space="PSUM") as a_ps,
            tc.tile_pool(name="y_ps", bufs=2, space="PSUM") as y_ps,
            tc.tile_pool(name="z_ps", bufs=1, space="PSUM") as z_ps,
        ):
            # ---- constants ----
            wt_sb = const.tile([128, 8, 1536], F32R)
            nc.sync.dma_start(out=wt_sb, in_=wt_r)
            wo_sb = const.tile([64, 8, 1024], F32R)
            nc.sync.dma_start(out=wo_sb, in_=wo_r)
            m0_sb = const.tile([128, 256], F32)
            nc.sync.dma_start(out=m0_sb, in_=m0d[:, :])
            zm_sb = const.tile([128, 32], F32R)
            nc.sync.dma_start(out=zm_sb, in_=zmd[:, :])
            zb_sb = const.tile([8, 512], F32R)
            nc.sync.dma_start(out=zb_sb, in_=zbd[:, :])

            S_sb = const.tile([128, NP, 64], F32R)   # per-pair state (l-pair, m)
            nc.vector.memset(S_sb.bitcast(F32), 0.0)
            carry = const.tile([128, NP], F32)       # knorm running carry
            nc.vector.memset(carry, 0.0)
            eps = const.tile([128, 1], F32)
            nc.vector.memset(eps, 1e-6)

            for qi in range(NQ):
                qsl = ds(qi * CQ, CQ)
                xq = xqp.tile([128, 8, CQ], F32R, tag="xq")
                nc.sync.dma_start(out=xq, in_=xT_r[:, :, qsl])

                # ---- transposed projections: eq = exp(q/8), kexpT = exp(k/8) ----
                eq = qkp.tile([128, NP, CQ], F32R, tag="eq")
                kexpT = qkp.tile([128, NP, CQ], F32R, tag="kexpT")
                for p in range(NP):
                    ps_q = mm_ps.tile([128, CQ], F32, tag="mm")
                    for dc in range(8):
                        nc.tensor.matmul(
                            ps_q, lhsT=wt_sb[:, dc, ds(p * 128, 128)],
                            rhs=xq[:, dc, :], start=(dc == 0), stop=(dc == 7))
                    nc.scalar.activation(eq[:, p, :], ps_q, AF.Exp, scale=SCALE)
                    ps_k = mm_ps.tile([128, CQ], F32, tag="mm")
                    for dc in range(8):
                        nc.tensor.matmul(
                            ps_k, lhsT=wt_sb[:, dc, ds(512 + p * 128, 128)],
                            rhs=xq[:, dc, :], start=(dc == 0), stop=(dc == 7))
                    nc.scalar.activation(kexpT[:, p, :], ps_k, AF.Exp, scale=SCALE)

                # ---- Z = per-head sums of eq (via masked-ones matmuls) ----
                zp = z_ps.tile([8, CQ], F32, tag="zp")
                for p in range(NP):
                    nc.tensor.matmul(
                        zp, lhsT=zm_sb[:, ds(p * 8, 8)], rhs=eq[:, p, :],
                        start=(p == 0), stop=(p == 3), skip_group_check=True)
                zsb = zsbp.tile([8, CQ], F32R, tag="zsb")
                nc.scalar.copy(out=zsb, in_=zp)

                # ---- knorm scan, den = knorm*Z, qs = eq/den ----
                knq = knp.tile([128, NP, CQ], F32, tag="knq")
                for p in range(NP):
                    nc.vector.tensor_tensor_scan(
                        knq[:, p, :], data0=kexpT[:, p, :],
                        data1=eps.to_broadcast((128, CQ)),
                        initial=carry[:, ds(p, 1)], op0=OP.add, op1=OP.add)
                    nc.vector.tensor_copy(
                        out=carry[:, ds(p, 1)], in_=knq[:, p, ds(CQ - 1, 1)])
                    zbp = mm_ps.tile([128, CQ], F32, tag="mm")
                    nc.tensor.matmul(zbp, lhsT=zb_sb[:, ds(p * 128, 128)],
                                     rhs=zsb, start=True, stop=True)
                    nc.vector.tensor_tensor(
                        out=knq[:, p, :], in0=knq[:, p, :], in1=zbp, op=OP.mult)
                    # 1/den via exp(-log(den)) on ScalarE (custom-DVE recip
                    # ops fail this walrus build's ISA lowering)
                    nc.scalar.activation(knq[:, p, :], knq[:, p, :], AF.Ln)
                    nc.scalar.activation(knq[:, p, :], knq[:, p, :], AF.Exp,
                                         scale=-1.0)
                    nc.vector.tensor_tensor(
                        out=eq[:, p, :], in0=eq[:, p, :], in1=knq[:, p, :],
                        op=OP.mult)
                qs = eq  # renamed: eq now holds qs = eq / (Z * knorm)
                if debug:
                    sweeps.append(nc.sync.dma_start(
                        out=dbg_qs[qi], in_=qs.bitcast(F32)))
                    sweeps.append(nc.sync.dma_start(
                        out=dbg_kn[qi], in_=knq))

                # ---- natural projections: kexp-nat, v ----
                knat = natp.tile([128, 4, CQ], F32R, tag="knat")
                v = natp.tile([128, 4, CQ], F32R, tag="v")
                for tci in range(4):
                    ps_kn = mm_ps.tile([128, CQ], F32, tag="mm")
                    ps_v = mm_ps.tile([128, CQ], F32, tag="mm")
                    for dc in range(8):
                        lh = xq[:, dc, ds(tci * 128, 128)]
                        nc.tensor.matmul(ps_kn, lhsT=lh, rhs=wt_sb[:, dc, ds(512, 512)],
                                         start=(dc == 0), stop=(dc == 7))
                        nc.tensor.matmul(ps_v, lhsT=lh, rhs=wt_sb[:, dc, ds(1024, 512)],
                                         start=(dc == 0), stop=(dc == 7))
                    nc.scalar.activation(knat[:, tci, :], ps_kn, AF.Exp, scale=SCALE)
                    nc.scalar.copy(out=v[:, tci, :], in_=ps_v)

                # ---- attention chunks (CH=256) ----
                # yq: per-head Y^T, heads on partitions 0-63, 8 head slots
                yq = ytp.tile([64, 8, CQ], F32R, tag="yt")
                for ci in range(CQ // CH):
                    t0 = ci * CH
                    csl = ds(t0, CH)
                    for p in range(NP):
                        # A' = kexp_s @ qs_t^T  per head, per s-block
                        ap = a_ps.tile([128, 4, CH], F32, tag="ap")
                        for hh in range(2):          # head in pair
                            hs = ds(hh * 64, 64)
                            for sg in range(2):      # s-block
                                nc.tensor.matmul(
                                    ap[:, 2 * hh + sg, :],
                                    lhsT=kexpT[hs, p, ds(t0 + sg * 128, 128)],
                                    rhs=qs[hs, p, csl],
                                    start=True, stop=True,
                                    tile_position=(hh * 64, 0))
                        # masked eviction -> abar
                        ab = abp.tile([128, 2, 384], F32R, tag="ab")
                        for hh in range(2):
                            nc.vector.tensor_tensor(
                                out=ab[:, hh, ds(0, 256)], in0=ap[:, 2 * hh, :],
                                in1=m0_sb, op=OP.mult)
                            nc.vector.tensor_tensor(
                                out=ab[:, hh, ds(256, 128)],
                                in0=ap[:, 2 * hh + 1, ds(128, 128)],
                                in1=m0_sb[:, ds(0, 128)], op=OP.mult)
                        # Y^T per head: inter (S @ qs) + intra (v^T @ Abar)
                        yp = y_ps.tile([64, 2, CH], F32, tag="ych")
                        for hh in range(2):
                            hs = ds(hh * 64, 64)
                            nc.tensor.matmul(
                                yp[:, hh, :],
                                lhsT=S_sb[hs, p, :], rhs=qs[hs, p, csl],
                                start=True, stop=False,
                                skip_group_check=True)
                            nc.tensor.matmul(
                                yp[:, hh, :],
                                lhsT=v[:, 2 * ci, ds(p * 128 + hh * 64, 64)],
                                rhs=ab[:, hh, ds(0, 256)],
                                start=False, stop=False,
                                skip_group_check=True)
                            nc.tensor.matmul(
                                yp[:, hh, ds(128, 128)],
                                lhsT=v[:, 2 * ci + 1, ds(p * 128 + hh * 64, 64)],
                                rhs=ab[:, hh, ds(256, 128)],
                                start=False, stop=True,
                                skip_group_check=True)
                        # S update: per-chunk delta (contiguous matmul group;
                        # resuming a PSUM accumulation group after other
                        # matmuls corrupts/has faulted on this HW), then
                        # accumulate into S_sb on the vector engine.
                        dS = s_ps.tile([128, 128], F32, tag="ds")
                        for sg in range(2):
                            nc.tensor.matmul(
                                dS,
                                lhsT=knat[:, 2 * ci + sg, ds(p * 128, 128)],
                                rhs=v[:, 2 * ci + sg, ds(p * 128, 128)],
                                start=(sg == 0), stop=(sg == 1))
                        nc.vector.tensor_tensor(
                            out=S_sb[ds(0, 64), p, :], in0=S_sb[ds(0, 64), p, :],
                            in1=dS[ds(0, 64), ds(0, 64)], op=OP.add)
                        nc.vector.tensor_tensor(
                            out=S_sb[ds(64, 64), p, :], in0=S_sb[ds(64, 64), p, :],
                            in1=dS[ds(64, 64), ds(64, 64)], op=OP.add)
                        # evict Y per head
                        for hh in range(2):
                            nc.scalar.copy(out=yq[:, 2 * p + hh, csl],
                                           in_=yp[:, hh, :])
                    if debug:
                        sweeps.append(nc.sync.dma_start(
                            out=dbg_s[qi * 2 + ci], in_=S_sb.bitcast(F32)))

                if debug:
                    sweeps.append(nc.sync.dma_start(
                        out=dbg_y[qi], in_=yq.bitcast(F32)))

                # ---- output projection for this quarter (K=64 per head) ----
                for tci in range(4):
                    ob = obp.tile([128, 2, 512], F32, tag="ob")
                    for eh in range(2):
                        po = mm_ps.tile([128, CQ], F32, tag="mm")
                        for h in range(8):
                            nc.tensor.matmul(
                                po, lhsT=yq[:, h, ds(tci * 128, 128)],
                                rhs=wo_sb[:, h, ds(eh * 512, 512)],
                                start=(h == 0), stop=(h == 7))
                        nc.scalar.copy(out=ob[:, eh, :], in_=po)
                    d = nc.sync.dma_start(
                        out=out[ds(qi * CQ + tci * 128, 128), :],
                        in_=ob.rearrange("p a b -> p (a b)"))
                    sweeps.append(d)

            # clock sweep: make the SP engine observe everything so the
            # end-of-kernel drain needs (almost) no waits of its own.
            for d in sweeps:
                nop = nc.sync.nop()
                add_dep_helper(nop.ins, d.ins, sync=True, reason="sweep")

    drop_sem_isa(nc)
    split_excess_waits(nc)
    return nc


_STATE = {}


def _get_nc():
    if "nc" not in _STATE:
        _STATE["nc"] = build_bass()
    return _STATE["nc"]


def _host_inputs(x, w, w_out):
    x = np.ascontiguousarray(np.asarray(x, dtype=np.float32))
    w = np.ascontiguousarray(np.asarray(w, dtype=np.float32))
    w_out = np.ascontiguousarray(np.asarray(w_out, dtype=np.float32))

    m0 = (np.arange(256)[None, :] >= np.arange(128)[:, None]).astype(np.float32)
    zm = np.zeros((128, 32), dtype=np.float32)
    for p in range(4):
        zm[0:64, p * 8 + 2 * p] = 1.0
        zm[64:128, p * 8 + 2 * p + 1] = 1.0
    zb = np.zeros((8, 512), dtype=np.float32)
    for p in range(4):
        zb[2 * p, p * 128: p * 128 + 64] = 1.0
        zb[2 * p + 1, p * 128 + 64: p * 128 + 128] = 1.0

    xTs = [np.ascontiguousarray(x[b].T) for b in range(B)]
    ins = []
    for c in range(8):
        b, hg = divmod(c, 2)
        r0 = hg * 512
        wt_c = np.ascontiguousarray(
            np.concatenate(
                [w[r0:r0 + 512], w[1024 + r0:1024 + r0 + 512],
                 w[2048 + r0:2048 + r0 + 512]], axis=0).T)     # (1024, 1536)
        wo_c = np.ascontiguousarray(
            w_out[r0:r0 + 512].reshape(8, 64, D))              # per-head rows
        ins.append({"xt": xTs[b], "wt": wt_c, "wo": wo_c,
                    "m0": m0, "zm": zm, "zb": zb})
    return ins


def kernel(x, w, w_out):
    nc = _get_nc()
    ins = _host_inputs(x, w, w_out)
    res = run_bass_kernel_spmd(nc, ins, core_ids=list(range(8)))
    out = np.empty((B, T, D), dtype=np.float32)
    for b in range(B):
        out[b] = res.results[2 * b]["out"] + res.results[2 * b + 1]["out"]
    return out


# revision 16
# speedup vs baseline: 4835.6901x; 4835.6901x over previous
"""Causal latent (linear) attention kernel for 8 Trainium2 NeuronCores.

Sharding: core c handles batch b = c//2 and head-group hg = c%2 (8 of 16
heads).  The (B,H,L,L) scan state is independent per (b,h) so there are no
cross-device transfers; each core emits a partial (T, D) output (its 512
y-dims times w_out rows) and the host sums the two partials per batch.

Algorithm (chunked linear attention, chunk C=256):
    q,k,v = x @ w.T  (per-head L=64)
    eq = exp(q/8); kexp = exp(k/8); knorm = cumsum(kexp + 1e-6)
    qs = eq / (Z * knorm),  Z[t] = sum_l eq[t,l]   (per head)
    per chunk: A = kexp_c @ qs_c^T (masked s<=t)
               Y_c = qs_c @ S + A^T-contracted v_c ; S += kexp_c^T v_c
    out = (Y heads concat) @ w_out

All matmuls run in float32r (reduced-precision fp32 PE mode, ~1.6e-4 rel
err measured) which streams at 1 cycle/row for free dims >= 256.  f32r
matmuls cannot target column-offset PSUM partitions, so all M=64 outputs
land on partitions 0-63 and the output projection contracts per head (K=64).
"""

import numpy as np

import concourse.bass as bass
import concourse.tile as tile
from concourse import mybir
from concourse.bass import ds
from concourse.bass_utils import run_bass_kernel_spmd
from concourse.tile import add_dep_helper

F32 = mybir.dt.float32
F32R = mybir.dt.float32r
AF = mybir.ActivationFunctionType
OP = mybir.AluOpType

B, T, D = 4, 2048, 1024
H, L = 16, 64
NP = 4           # head-pairs per core
CQ = 512         # quarter (outer tile) size along T
NQ = T // CQ     # 4
CH = 256         # attention chunk
SCALE = 0.125    # 1/sqrt(L)


def drop_sem_isa(nc):
    """The end-of-kernel semaphore RANGE_CLEAR (InstISA op 176) fails walrus
    codegen ("ISA wrong length") for larger sem ranges in this build.  NRT
    re-initializes semaphore state per execution, so drop it (verified: 3
    repeated executions stay correct).  Its waits move onto a NoOp."""
    n = 0
    for f in nc.m.functions:
        for blk in f.blocks:
            keep = []
            for inst in blk.instructions:
                if type(inst).__name__ == "InstISA":
                    n += 1
                    si = inst.sync_info
                    if si is not None and si.on_wait:
                        nop = mybir.InstNoOp(name=f"{inst.name}-del", ins=[], outs=[])
                        nop.engine = inst.engine
                        nop.sync_info = si
                        keep.append(nop)
                    continue
                keep.append(inst)
            blk.instructions = keep
    return n


def split_excess_waits(nc):
    """This walrus build accepts only ONE sync-wait command per instruction.
    Move excess waits onto same-engine NoOps inserted just before."""
    n = 0
    for f in nc.m.functions:
        for blk in f.blocks:
            new_insts = []
            for inst in blk.instructions:
                si = inst.sync_info
                waits = list(si.on_wait) if si is not None else []
                if len(waits) > 1:
                    for i, wchunk in enumerate(waits[:-1]):
                        nop = mybir.InstNoOp(name=f"{inst.name}-ws{i}", ins=[], outs=[])
                        nop.engine = inst.engine
                        nop.sync_info = mybir.SyncInfo(on_wait=[wchunk], on_update=[])
                        new_insts.append(nop)
                        n += 1
                    inst.sync_info = mybir.SyncInfo(
                        on_wait=waits[-1:], on_update=list(si.on_update)
                    )
                new_insts.append(inst)
            blk.instructions = new_insts
    return n


def build_bass(debug=False, reps=1):
    nc = bass.Bass(trn_type="TRN2")

    xT = nc.dram_tensor("xt", [D, T], F32R, kind="ExternalInput")        # x[b].T
    wt = nc.dram_tensor("wt", [D, 1536], F32R, kind="ExternalInput")     # [q|k|v] cols
    wo = nc.dram_tensor("wo", [8, 64, D], F32R, kind="ExternalInput")    # per-head rows
    m0d = nc.dram_tensor("m0", [128, 256], F32, kind="ExternalInput")    # causal mask
    zmd = nc.dram_tensor("zm", [128, 32], F32R, kind="ExternalInput")    # Z-sum lhsT
    zbd = nc.dram_tensor("zb", [8, 512], F32R, kind="ExternalInput")     # Z-bcast lhsT
    out = nc.dram_tensor("out", [T, D], F32, kind="ExternalOutput")
    if debug:
        dbg_qs = nc.dram_tensor("dbg_qs", [NQ, 128, NP, CQ], F32, kind="ExternalOutput")
        dbg_kn = nc.dram_tensor("dbg_kn", [NQ, 128, NP, CQ], F32, kind="ExternalOutput")
        dbg_s = nc.dram_tensor("dbg_s", [NQ * 2, 128, NP, 64], F32, kind="ExternalOutput")
        dbg_y = nc.dram_tensor("dbg_y", [NQ, 64, 8, CQ], F32, kind="ExternalOutput")

    xT_r = xT[:, :].rearrange("(o p) t -> p o t", p=128)
    wt_r = wt[:, :].rearrange("(o p) j -> p o j", p=128)
    wo_r = wo[:, :, :].rearrange("h p e -> p h e")

    sweeps = []  # instructions the final clock-sweep nops must cover

    with tile.TileContext(nc) as tc:
        with (
            tc.tile_pool(name="const", bufs=1) as const,
            tc.tile_pool(name="xq", bufs=2) as xqp,
            tc.tile_pool(name="qk", bufs=2) as qkp,
            tc.tile_pool(name="kn", bufs=1) as knp,
            tc.tile_pool(name="natp", bufs=1) as natp,
            tc.tile_pool(name="abar", bufs=2) as abp,
            tc.tile_pool(name="yt", bufs=1) as ytp,
            tc.tile_pool(name="zsb", bufs=1) as zsbp,
            tc.tile_pool(name="ob", bufs=2) as obp,
            tc.tile_pool(name="s_ps", bufs=1, space="PSUM") as s_ps,
            tc.tile_pool(name="mm_ps", bufs=2, space="PSUM") as mm_ps,
            tc.tile_pool(name="a_ps", bufs=1, space="PSUM") as a_ps,
            tc.tile_pool(name="y_ps", bufs=2, space="PSUM") as y_ps,
            tc.tile_pool(name="z_ps", bufs=1, space="PSUM") as z_ps,
        ):
            # ---- constants ----
            wt_sb = const.tile([128, 8, 1536], F32R)
            nc.sync.dma_start(out=wt_sb, in_=wt_r)
            wo_sb = const.tile([64, 8, 1024], F32R)
            nc.sync.dma_start(out=wo_sb, in_=wo_r)
            m0_sb = const.tile([128, 256], F32)
            nc.sync.dma_start(out=m0_sb, in_=m0d[:, :])
            zm_sb = const.tile([128, 32], F32R)
            nc.sync.dma_start(out=zm_sb, in_=zmd[:, :])
            zb_sb = const.tile([8, 512], F32R)
            nc.sync.dma_start(out=zb_sb, in_=zbd[:, :])

            S_sb = const.tile([128, NP, 64], F32R)   # per-pair state (l-pair, m)
            nc.vector.memset(S_sb.bitcast(F32), 0.0)
            carry = const.tile([128, NP], F32)       # knorm running carry
            nc.vector.memset(carry, 0.0)
            eps = const.tile([128, 1], F32)
            nc.vector.memset(eps, 1e-6)

            for rep in range(reps):
              if rep > 0:
                nc.vector.memset(S_sb.bitcast(F32), 0.0)
                nc.vector.memset(carry, 0.0)
              for qi in range(NQ):
                qsl = ds(qi * CQ, CQ)
                xq = xqp.tile([128, 8, CQ], F32R, tag="xq")
                nc.sync.dma_start(out=xq, in_=xT_r[:, :, qsl])

                # ---- transposed projections: eq = exp(q/8), kexpT = exp(k/8) ----
                eq = qkp.tile([128, NP, CQ], F32R, tag="eq")
                kexpT = qkp.tile([128, NP, CQ], F32R, tag="kexpT")
                for p in range(NP):
                    ps_q = mm_ps.tile([128, CQ], F32, tag="mm")
                    for dc in range(8):
                        nc.tensor.matmul(
                            ps_q, lhsT=wt_sb[:, dc, ds(p * 128, 128)],
                            rhs=xq[:, dc, :], start=(dc == 0), stop=(dc == 7))
                    nc.scalar.activation(eq[:, p, :], ps_q, AF.Exp, scale=SCALE)
                    ps_k = mm_ps.tile([128, CQ], F32, tag="mm")
                    for dc in range(8):
                        nc.tensor.matmul(
                            ps_k, lhsT=wt_sb[:, dc, ds(512 + p * 128, 128)],
                            rhs=xq[:, dc, :], start=(dc == 0), stop=(dc == 7))
                    nc.scalar.activation(kexpT[:, p, :], ps_k, AF.Exp, scale=SCALE)

                # ---- Z = per-head sums of eq (via masked-ones matmuls) ----
                zp = z_ps.tile([8, CQ], F32, tag="zp")
                for p in range(NP):
                    nc.tensor.matmul(
                        zp, lhsT=zm_sb[:, ds(p * 8, 8)], rhs=eq[:, p, :],
                        start=(p == 0), stop=(p == 3), skip_group_check=True)
                zsb = zsbp.tile([8, CQ], F32R, tag="zsb")
                nc.scalar.copy(out=zsb, in_=zp)

                # ---- knorm scan, den = knorm*Z, qs = eq/den ----
                knq = knp.tile([128, NP, CQ], F32, tag="knq")
                for p in range(NP):
                    nc.vector.tensor_tensor_scan(
                        knq[:, p, :], data0=kexpT[:, p, :],
                        data1=eps.to_broadcast((128, CQ)),
                        initial=carry[:, ds(p, 1)], op0=OP.add, op1=OP.add)
                    nc.vector.tensor_copy(
                        out=carry[:, ds(p, 1)], in_=knq[:, p, ds(CQ - 1, 1)])
                    zbp = mm_ps.tile([128, CQ], F32, tag="mm")
                    nc.tensor.matmul(zbp, lhsT=zb_sb[:, ds(p * 128, 128)],
                                     rhs=zsb, start=True, stop=True)
                    nc.vector.tensor_tensor(
                        out=knq[:, p, :], in0=knq[:, p, :], in1=zbp, op=OP.mult)
                    # 1/den via exp(-log(den)) on ScalarE (custom-DVE recip
                    # ops fail this walrus build's ISA lowering)
                    nc.scalar.activation(knq[:, p, :], knq[:, p, :], AF.Ln)
                    nc.scalar.activation(knq[:, p, :], knq[:, p, :], AF.Exp,
                                         scale=-1.0)
                    nc.vector.tensor_tensor(
                        out=eq[:, p, :], in0=eq[:, p, :], in1=knq[:, p, :],
                        op=OP.mult)
                qs = eq  # renamed: eq now holds qs = eq / (Z * knorm)
                if debug:
                    sweeps.append(nc.sync.dma_start(
                        out=dbg_qs[qi], in_=qs.bitcast(F32)))
                    sweeps.append(nc.sync.dma_start(
                        out=dbg_kn[qi], in_=knq))

                # ---- natural projections: kexp-nat, v ----
                knat = natp.tile([128, 4, CQ], F32R, tag="knat")
                v = natp.tile([128, 4, CQ], F32R, tag="v")
                for tci in range(4):
                    ps_kn = mm_ps.tile([128, CQ], F32, tag="mm")
                    ps_v = mm_ps.tile([128, CQ], F32, tag="mm")
                    for dc in range(8):
                        lh = xq[:, dc, ds(tci * 128, 128)]
                        nc.tensor.matmul(ps_kn, lhsT=lh, rhs=wt_sb[:, dc, ds(512, 512)],
                                         start=(dc == 0), stop=(dc == 7))
                        nc.tensor.matmul(ps_v, lhsT=lh, rhs=wt_sb[:, dc, ds(1024, 512)],
                                         start=(dc == 0), stop=(dc == 7))
                    nc.scalar.activation(knat[:, tci, :], ps_kn, AF.Exp, scale=SCALE)
                    nc.scalar.copy(out=v[:, tci, :], in_=ps_v)

                # ---- attention chunks (CH=256) ----
                # yq: per-head Y^T, heads on partitions 0-63, 8 head slots
                yq = ytp.tile([64, 8, CQ], F32R, tag="yt")
                for ci in range(CQ // CH):
                    t0 = ci * CH
                    csl = ds(t0, CH)
                    for p in range(NP):
                        # A' = kexp_s @ qs_t^T  per head, per s-block
                        ap = a_ps.tile([128, 4, CH], F32, tag="ap")
                        for hh in range(2):          # head in pair
                            hs = ds(hh * 64, 64)
                            for sg in range(2):      # s-block
                                nc.tensor.matmul(
                                    ap[:, 2 * hh + sg, :],
                                    lhsT=kexpT[hs, p, ds(t0 + sg * 128, 128)],
                                    rhs=qs[hs, p, csl],
                                    start=True, stop=True,
                                    tile_position=(hh * 64, 0))
                        # masked eviction -> abar
                        ab = abp.tile([128, 2, 384], F32R, tag="ab")
                        for hh in range(2):
                            nc.vector.tensor_tensor(
                                out=ab[:, hh, ds(0, 256)], in0=ap[:, 2 * hh, :],
                                in1=m0_sb, op=OP.mult)
                            nc.vector.tensor_tensor(
                                out=ab[:, hh, ds(256, 128)],
                                in0=ap[:, 2 * hh + 1, ds(128, 128)],
                                in1=m0_sb[:, ds(0, 128)], op=OP.mult)
                        # Y^T per head: inter (S @ qs) + intra (v^T @ Abar)
                        yp = y_ps.tile([64, 2, CH], F32, tag="ych")
                        for hh in range(2):
                            hs = ds(hh * 64, 64)
                            nc.tensor.matmul(
                                yp[:, hh, :],
                                lhsT=S_sb[hs, p, :], rhs=qs[hs, p, csl],
                                start=True, stop=False,
                                skip_group_check=True)
                            nc.tensor.matmul(
                                yp[:, hh, :],
                                lhsT=v[:, 2 * ci, ds(p * 128 + hh * 64, 64)],
                                rhs=ab[:, hh, ds(0, 256)],
                                start=False, stop=False,
                                skip_group_check=True)
                            nc.tensor.matmul(
                                yp[:, hh, ds(128, 128)],
                                lhsT=v[:, 2 * ci + 1, ds(p * 128 + hh * 64, 64)],
                                rhs=ab[:, hh, ds(256, 128)],
                                start=False, stop=True,
                                skip_group_check=True)
                        # S update: per-chunk delta (contiguous matmul group;
                        # resuming a PSUM accumulation group after other
                        # matmuls corrupts/has faulted on this HW), then
                        # accumulate into S_sb on the vector engine.
                        dS = s_ps.tile([128, 128], F32, tag="ds")
                        for sg in range(2):
                            nc.tensor.matmul(
                                dS,
                                lhsT=knat[:, 2 * ci + sg, ds(p * 128, 128)],
                                rhs=v[:, 2 * ci + sg, ds(p * 128, 128)],
                                start=(sg == 0), stop=(sg == 1))
                        nc.vector.tensor_tensor(
                            out=S_sb[ds(0, 64), p, :], in0=S_sb[ds(0, 64), p, :],
                            in1=dS[ds(0, 64), ds(0, 64)], op=OP.add)
                        nc.vector.tensor_tensor(
                            out=S_sb[ds(64, 64), p, :], in0=S_sb[ds(64, 64), p, :],
                            in1=dS[ds(64, 64), ds(64, 64)], op=OP.add)
                        # evict Y per head
                        for hh in range(2):
                            nc.scalar.copy(out=yq[:, 2 * p + hh, csl],
                                           in_=yp[:, hh, :])
                    if debug:
                        sweeps.append(nc.sync.dma_start(
                            out=dbg_s[qi * 2 + ci], in_=S_sb.bitcast(F32)))

                if debug:
                    sweeps.append(nc.sync.dma_start(
                        out=dbg_y[qi], in_=yq.bitcast(F32)))

                # ---- output projection for this quarter (K=64 per head) ----
                for tci in range(4):
                    ob = obp.tile([128, 2, 512], F32, tag="ob")
                    for eh in range(2):
                        po = mm_ps.tile([128, CQ], F32, tag="mm")
                        for h in range(8):
                            nc.tensor.matmul(
                                po, lhsT=yq[:, h, ds(tci * 128, 128)],
                                rhs=wo_sb[:, h, ds(eh * 512, 512)],
                                start=(h == 0), stop=(h == 7))
                        nc.scalar.copy(out=ob[:, eh, :], in_=po)
                    d = nc.sync.dma_start(
                        out=out[ds(qi * CQ + tci * 128, 128), :],
                        in_=ob.rearrange("p a b -> p (a b)"))
                    sweeps.append(d)

            # clock sweep: make the SP engine observe everything so the
            # end-of-kernel drain needs (almost) no waits of its own.
            for d in sweeps:
                nop = nc.sync.nop()
                add_dep_helper(nop.ins, d.ins, sync=True, reason="sweep")

    drop_sem_isa(nc)
    split_excess_waits(nc)
    return nc


_STATE = {}


def _get_nc():
    if "nc" not in _STATE:
        _STATE["nc"] = build_bass()
    return _STATE["nc"]


def _host_inputs(x, w, w_out):
    x = np.ascontiguousarray(np.asarray(x, dtype=np.float32))
    w = np.ascontiguousarray(np.asarray(w, dtype=np.float32))
    w_out = np.ascontiguousarray(np.asarray(w_out, dtype=np.float32))

    m0 = (np.arange(256)[None, :] >= np.arange(128)[:, None]).astype(np.float32)
    zm = np.zeros((128, 32), dtype=np.float32)
    for p in range(4):
        zm[0:64, p * 8 + 2 * p] = 1.0
        zm[64:128, p * 8 + 2 * p + 1] = 1.0
    zb = np.zeros((8, 512), dtype=np.float32)
    for p in range(4):
        zb[2 * p, p * 128: p * 128 + 64] = 1.0
        zb[2 * p + 1, p * 128 + 64: p * 128 + 128] = 1.0

    xTs = [np.ascontiguousarray(x[b].T) for b in range(B)]
    ins = []
    for c in range(8):
        b, hg = divmod(c, 2)
        r0 = hg * 512
        wt_c = np.ascontiguousarray(
            np.concatenate(
                [w[r0:r0 + 512], w[1024 + r0:1024 + r0 + 512],
                 w[2048 + r0:2048 + r0 + 512]], axis=0).T)     # (1024, 1536)
        wo_c = np.ascontiguousarray(
            w_out[r0:r0 + 512].reshape(8, 64, D))              # per-head rows
        ins.append({"xt": xTs[b], "wt": wt_c, "wo": wo_c,
                    "m0": m0, "zm": zm, "zb": zb})
    return ins


def kernel(x, w, w_out):
    nc = _get_nc()
    ins = _host_inputs(x, w, w_out)
    res = run_bass_kernel_spmd(nc, ins, core_ids=list(range(8)))
    out = np.empty((B, T, D), dtype=np.float32)
    for b in range(B):
        out[b] = res.results[2 * b]["out"] + res.results[2 * b + 1]["out"]
    return out


# revision 30
# speedup vs baseline: 6457.9878x; 1.3355x over previous
"""Causal latent (linear) attention kernel for 8 Trainium2 NeuronCores.

Sharding: core c handles batch b = c//2 and head-group hg = c%2 (8 of 16
heads).  The (B,H,L,L) scan state is independent per (b,h) so there are no
cross-device transfers; each core emits a partial (T, D) output (its 512
y-dims times w_out rows) and the host sums the two partials per batch.

Algorithm (chunked linear attention, chunk C=256):
    q,k,v = x @ w.T  (per-head L=64)
    eq = exp(q/8); kexp = exp(k/8); knorm = cumsum(kexp + 1e-6)
    qs = eq / (Z * knorm),  Z[t] = sum_l eq[t,l]   (per head)
    per chunk: A = kexp_c @ qs_c^T (masked s<=t)
               Y_c = qs_c @ S + A^T-contracted v_c ; S += kexp_c^T v_c
    out = (Y heads concat) @ w_out

All matmuls run in float32r (reduced-precision fp32 PE mode, ~1.6e-4 rel
err measured) which streams at 1 cycle/row for free dims >= 256.  f32r
matmuls cannot target column-offset PSUM partitions, so all M=64 outputs
land on partitions 0-63 and the output projection contracts per head (K=64).
"""

import numpy as np

import concourse.bass as bass
import concourse.tile as tile
from concourse import mybir
from concourse.bass import ds
from concourse.bass_utils import run_bass_kernel_spmd
from concourse.tile import add_dep_helper

F32 = mybir.dt.float32
F32R = mybir.dt.float32r
AF = mybir.ActivationFunctionType
OP = mybir.AluOpType

B, T, D = 4, 2048, 1024
H, L = 16, 64
NP = 4           # head-pairs per core
CQ = 512         # quarter (outer tile) size along T
NQ = T // CQ     # 4
CH = 256         # attention chunk
SCALE = 0.125    # 1/sqrt(L)


def drop_sem_isa(nc):
    """The end-of-kernel semaphore RANGE_CLEAR (InstISA op 176) fails walrus
    codegen ("ISA wrong length") for larger sem ranges in this build.  NRT
    re-initializes semaphore state per execution, so drop it (verified: 3
    repeated executions stay correct).  Its waits move onto a NoOp."""
    n = 0
    for f in nc.m.functions:
        for blk in f.blocks:
            keep = []
            for inst in blk.instructions:
                if type(inst).__name__ == "InstISA":
                    n += 1
                    si = inst.sync_info
                    if si is not None and si.on_wait:
                        nop = mybir.InstNoOp(name=f"{inst.name}-del", ins=[], outs=[])
                        nop.engine = inst.engine
                        nop.sync_info = si
                        keep.append(nop)
                    continue
                keep.append(inst)
            blk.instructions = keep
    return n


def split_excess_waits(nc):
    """This walrus build accepts only ONE sync-wait command per instruction.
    Move excess waits onto same-engine NoOps inserted just before."""
    n = 0
    for f in nc.m.functions:
        for blk in f.blocks:
            new_insts = []
            for inst in blk.instructions:
                si = inst.sync_info
                waits = list(si.on_wait) if si is not None else []
                if len(waits) > 1:
                    for i, wchunk in enumerate(waits[:-1]):
                        nop = mybir.InstNoOp(name=f"{inst.name}-ws{i}", ins=[], outs=[])
                        nop.engine = inst.engine
                        nop.sync_info = mybir.SyncInfo(on_wait=[wchunk], on_update=[])
                        new_insts.append(nop)
                        n += 1
                    inst.sync_info = mybir.SyncInfo(
                        on_wait=waits[-1:], on_update=list(si.on_update)
                    )
                new_insts.append(inst)
            blk.instructions = new_insts
    return n


def build_bass(debug=False, reps=1):
    nc = bass.Bass(trn_type="TRN2")

    xT = nc.dram_tensor("xt", [D, T], F32R, kind="ExternalInput")        # x[b].T
    wt = nc.dram_tensor("wt", [D, 1536], F32R, kind="ExternalInput")     # [q|k|v] cols
    wo = nc.dram_tensor("wo", [8, 64, D], F32R, kind="ExternalInput")    # per-head rows
    m0d = nc.dram_tensor("m0", [128, 512], F32, kind="ExternalInput")    # [U|1|0|U] masks
    zmd = nc.dram_tensor("zm", [128, 32], F32R, kind="ExternalInput")    # Z-sum lhsT
    zbd = nc.dram_tensor("zb", [8, 512], F32R, kind="ExternalInput")     # Z-bcast lhsT
    out = nc.dram_tensor("out", [T, D], F32, kind="ExternalOutput")
    if debug:
        dbg_qs = nc.dram_tensor("dbg_qs", [NQ, 128, NP, CQ], F32, kind="ExternalOutput")
        dbg_kn = nc.dram_tensor("dbg_kn", [NQ, 128, NP, CQ], F32, kind="ExternalOutput")
        dbg_s = nc.dram_tensor("dbg_s", [NQ * 2, 128, NP, 64], F32, kind="ExternalOutput")
        dbg_y = nc.dram_tensor("dbg_y", [NQ, 64, 8, CQ], F32, kind="ExternalOutput")

    xT_r = xT[:, :].rearrange("(o p) t -> p o t", p=128)
    wt_r = wt[:, :].rearrange("(o p) j -> p o j", p=128)
    wo_r = wo[:, :, :].rearrange("h p e -> p h e")

    sweeps = []  # instructions the final clock-sweep nops must cover

    with tile.TileContext(nc) as tc:
        with (
            tc.tile_pool(name="const", bufs=1) as const,
            tc.tile_pool(name="xq", bufs=2) as xqp,
            tc.tile_pool(name="qk", bufs=2) as qkp,
            tc.tile_pool(name="kn", bufs=1) as knp,
            tc.tile_pool(name="natp", bufs=1) as natp,
            tc.tile_pool(name="abar", bufs=2) as abp,
            tc.tile_pool(name="yt", bufs=1) as ytp,
            tc.tile_pool(name="zsb", bufs=1) as zsbp,
            tc.tile_pool(name="ob", bufs=2) as obp,
            tc.tile_pool(name="s_ps", bufs=1, space="PSUM") as s_ps,
            tc.tile_pool(name="mm_ps", bufs=2, space="PSUM") as mm_ps,
            tc.tile_pool(name="a_ps", bufs=2, space="PSUM") as a_ps,
            tc.tile_pool(name="y_ps", bufs=2, space="PSUM") as y_ps,
            tc.tile_pool(name="z_ps", bufs=1, space="PSUM") as z_ps,
        ):
            # ---- constants ----
            wt_sb = const.tile([128, 8, 1536], F32R)
            nc.sync.dma_start(out=wt_sb, in_=wt_r)
            wo_sb = const.tile([64, 8, 1024], F32R)
            nc.sync.dma_start(out=wo_sb, in_=wo_r)
            m0_sb = const.tile([128, 512], F32)
            nc.sync.dma_start(out=m0_sb, in_=m0d[:, :])
            zm_sb = const.tile([128, 32], F32R)
            nc.sync.dma_start(out=zm_sb, in_=zmd[:, :])
            zb_sb = const.tile([8, 512], F32R)
            nc.sync.dma_start(out=zb_sb, in_=zbd[:, :])

            S_sb = const.tile([128, NP, 64], F32R)   # per-pair state (l-pair, m)
            nc.vector.memset(S_sb.bitcast(F32), 0.0)
            carry = const.tile([128, NP], F32)       # knorm running carry
            nc.vector.memset(carry, 0.0)
            eps = const.tile([128, 1], F32)
            nc.vector.memset(eps, 1e-6)

            for rep in range(reps):
              if rep > 0:
                nc.vector.memset(S_sb.bitcast(F32), 0.0)
                nc.vector.memset(carry, 0.0)
              for qi in range(NQ):
                qsl = ds(qi * CQ, CQ)
                xq = xqp.tile([128, 8, CQ], F32R, tag="xq")
                nc.sync.dma_start(out=xq, in_=xT_r[:, :, qsl])

                # ---- transposed projections: eq = exp(q/8), kexpT = exp(k/8) ----
                eq = qkp.tile([128, NP, CQ], F32R, tag="eq")
                kexpT = qkp.tile([128, NP, CQ], F32R, tag="kexpT")
                for p in range(NP):
                    ps_q = mm_ps.tile([128, CQ], F32, tag="mm")
                    for dc in range(8):
                        nc.tensor.matmul(
                            ps_q, lhsT=wt_sb[:, dc, ds(p * 128, 128)],
                            rhs=xq[:, dc, :], start=(dc == 0), stop=(dc == 7))
                    nc.scalar.activation(eq[:, p, :], ps_q, AF.Exp, scale=SCALE)
                    ps_k = mm_ps.tile([128, CQ], F32, tag="mm")
                    for dc in range(8):
                        nc.tensor.matmul(
                            ps_k, lhsT=wt_sb[:, dc, ds(512 + p * 128, 128)],
                            rhs=xq[:, dc, :], start=(dc == 0), stop=(dc == 7))
                    nc.scalar.activation(kexpT[:, p, :], ps_k, AF.Exp, scale=SCALE)

                # ---- Z = per-head sums of eq (via masked-ones matmuls) ----
                zp = z_ps.tile([8, CQ], F32, tag="zp")
                for p in range(NP):
                    nc.tensor.matmul(
                        zp, lhsT=zm_sb[:, ds(p * 8, 8)], rhs=eq[:, p, :],
                        start=(p == 0), stop=(p == 3), skip_group_check=True)
                zsb = zsbp.tile([8, CQ], F32R, tag="zsb")
                nc.scalar.copy(out=zsb, in_=zp)

                # ---- knorm scan, den = knorm*Z, qs = eq/den ----
                knq = knp.tile([128, NP, CQ], F32, tag="knq")
                for p in range(NP):
                    nc.vector.tensor_tensor_scan(
                        knq[:, p, :], data0=kexpT[:, p, :],
                        data1=eps.to_broadcast((128, CQ)),
                        initial=carry[:, ds(p, 1)], op0=OP.add, op1=OP.add)
                    nc.vector.tensor_copy(
                        out=carry[:, ds(p, 1)], in_=knq[:, p, ds(CQ - 1, 1)])
                    zbp = mm_ps.tile([128, CQ], F32, tag="mm")
                    nc.tensor.matmul(zbp, lhsT=zb_sb[:, ds(p * 128, 128)],
                                     rhs=zsb, start=True, stop=True)
                    nc.vector.tensor_tensor(
                        out=knq[:, p, :], in0=knq[:, p, :], in1=zbp, op=OP.mult)
                    # 1/den via exp(-log(den)) on ScalarE (custom-DVE recip
                    # ops fail this walrus build's ISA lowering)
                    nc.scalar.activation(knq[:, p, :], knq[:, p, :], AF.Ln)
                    nc.scalar.activation(knq[:, p, :], knq[:, p, :], AF.Exp,
                                         scale=-1.0)
                    nc.vector.tensor_tensor(
                        out=eq[:, p, :], in0=eq[:, p, :], in1=knq[:, p, :],
                        op=OP.mult)
                qs = eq  # renamed: eq now holds qs = eq / (Z * knorm)
                if debug:
                    sweeps.append(nc.sync.dma_start(
                        out=dbg_qs[qi], in_=qs.bitcast(F32)))
                    sweeps.append(nc.sync.dma_start(
                        out=dbg_kn[qi], in_=knq))

                # ---- natural projections: kexp-nat, v ----
                knat = natp.tile([128, 4, CQ], F32R, tag="knat")
                v = natp.tile([128, 4, CQ], F32R, tag="v")
                for tci in range(4):
                    ps_kn = mm_ps.tile([128, CQ], F32, tag="mm")
                    ps_v = mm_ps.tile([128, CQ], F32, tag="mm")
                    for dc in range(8):
                        lh = xq[:, dc, ds(tci * 128, 128)]
                        nc.tensor.matmul(ps_kn, lhsT=lh, rhs=wt_sb[:, dc, ds(512, 512)],
                                         start=(dc == 0), stop=(dc == 7))
                        nc.tensor.matmul(ps_v, lhsT=lh, rhs=wt_sb[:, dc, ds(1024, 512)],
                                         start=(dc == 0), stop=(dc == 7))
                    nc.scalar.activation(knat[:, tci, :], ps_kn, AF.Exp, scale=SCALE)
                    nc.scalar.copy(out=v[:, tci, :], in_=ps_v)

                # ---- attention chunks (CH=256) ----
                # yq: per-head Y^T, heads on partitions 0-63, 8 head slots
                yq = ytp.tile([64, 8, CQ], F32R, tag="yt")
                for ci in range(CQ // CH):
                    t0 = ci * CH
                    csl = ds(t0, CH)
                    for p in range(NP):
                        # A' = kexp_s @ qs_t^T  per head, per s-block;
                        # per-head psum tiles for finer pipelining
                        ab = abp.tile([128, 2, 512], F32R, tag="ab")
                        for hh in range(2):          # head in pair
                            hs = ds(hh * 64, 64)
                            ap = a_ps.tile([128, 2, CH], F32, tag="ap")
                            for sg in range(2):      # s-block
                                nc.tensor.matmul(
                                    ap[:, sg, :],
                                    lhsT=kexpT[hs, p, ds(t0 + sg * 128, 128)],
                                    rhs=qs[hs, p, csl],
                                    start=True, stop=True,
                                    tile_position=(hh * 64, 0))
                            # masked eviction -> abar (sg1 stored full-width,
                            # zero left half, so intra stays on N>=256 lane)
                            nc.vector.tensor_tensor(
                                out=ab[:, hh, ds(0, 256)], in0=ap[:, 0, :],
                                in1=m0_sb[:, ds(0, 256)], op=OP.mult)
                            nc.vector.tensor_tensor(
                                out=ab[:, hh, ds(256, 256)],
                                in0=ap[:, 1, :],
                                in1=m0_sb[:, ds(256, 256)], op=OP.mult)
                        # Y^T per head: inter (S @ qs) + intra (v^T @ Abar)
                        yp = y_ps.tile([64, 2, CH], F32, tag="ych")
                        for hh in range(2):
                            hs = ds(hh * 64, 64)
                            nc.tensor.matmul(
                                yp[:, hh, :],
                                lhsT=S_sb[hs, p, :], rhs=qs[hs, p, csl],
                                start=True, stop=False,
                                skip_group_check=True)
                            nc.tensor.matmul(
                                yp[:, hh, :],
                                lhsT=v[:, 2 * ci, ds(p * 128 + hh * 64, 64)],
                                rhs=ab[:, hh, ds(0, 256)],
                                start=False, stop=False,
                                skip_group_check=True)
                            nc.tensor.matmul(
                                yp[:, hh, :],
                                lhsT=v[:, 2 * ci + 1, ds(p * 128 + hh * 64, 64)],
                                rhs=ab[:, hh, ds(256, 256)],
                                start=False, stop=True,
                                skip_group_check=True)
                        nc.scalar.copy(out=yq[:, 2 * p: 2 * p + 2, csl],
                                       in_=yp[:, :, :])
                    # S update: per-chunk deltas.  rhs spans TWO pairs' v
                    # columns (N=256 -> f32r fast lane); each pair's own
                    # lhsT writes its own psum region, cross-pair columns
                    # are garbage and never read.  Groups stay contiguous
                    # (resumed PSUM groups corrupt on this HW).
                    for pg in range(2):
                        dS = s_ps.tile([128, 2, 256], F32, tag="ds")
                        for pp in range(2):
                            for sg in range(2):
                                nc.tensor.matmul(
                                    dS[:, pp, :],
                                    lhsT=knat[:, 2 * ci + sg,
                                              ds((2 * pg + pp) * 128, 128)],
                                    rhs=v[:, 2 * ci + sg, ds(pg * 256, 256)],
                                    start=(sg == 0), stop=(sg == 1))
                        for pp in range(2):
                            p2 = 2 * pg + pp
                            m0_ = pp * 128
                            nc.vector.tensor_tensor(
                                out=S_sb[ds(0, 64), p2, :],
                                in0=S_sb[ds(0, 64), p2, :],
                                in1=dS[ds(0, 64), pp, ds(m0_, 64)], op=OP.add)
                            nc.vector.tensor_tensor(
                                out=S_sb[ds(64, 64), p2, :],
                                in0=S_sb[ds(64, 64), p2, :],
                                in1=dS[ds(64, 64), pp, ds(m0_ + 64, 64)], op=OP.add)
                    if debug:
                        sweeps.append(nc.sync.dma_start(
                            out=dbg_s[qi * 2 + ci], in_=S_sb.bitcast(F32)))

                if debug:
                    sweeps.append(nc.sync.dma_start(
                        out=dbg_y[qi], in_=yq.bitcast(F32)))

                # ---- output projection for this quarter (K=64 per head) ----
                for tci in range(4):
                    ob = obp.tile([128, 2, 512], F32, tag="ob")
                    for eh in range(2):
                        po = mm_ps.tile([128, CQ], F32, tag="mm")
                        for h in range(8):
                            nc.tensor.matmul(
                                po, lhsT=yq[:, h, ds(tci * 128, 128)],
                                rhs=wo_sb[:, h, ds(eh * 512, 512)],
                                start=(h == 0), stop=(h == 7))
                        nc.scalar.copy(out=ob[:, eh, :], in_=po)
                    d = nc.sync.dma_start(
                        out=out[ds(qi * CQ + tci * 128, 128), :],
                        in_=ob.rearrange("p a b -> p (a b)"))
                    sweeps.append(d)

            # clock sweep: make the SP engine observe everything so the
            # end-of-kernel drain needs (almost) no waits of its own.
            for d in sweeps:
                nop = nc.sync.nop()
                add_dep_helper(nop.ins, d.ins, sync=True, reason="sweep")

    drop_sem_isa(nc)
    split_excess_waits(nc)
    return nc


_STATE = {}


def _get_nc():
    if "nc" not in _STATE:
        _STATE["nc"] = build_bass()
    return _STATE["nc"]


def _host_inputs(x, w, w_out):
    x = np.ascontiguousarray(np.asarray(x, dtype=np.float32))
    w = np.ascontiguousarray(np.asarray(w, dtype=np.float32))
    w_out = np.ascontiguousarray(np.asarray(w_out, dtype=np.float32))

    # masks: cols 0-255 for s-block 0 ([U|1]); cols 256-511 for s-block 1
    # ([0|U]) stored full-width so the sg1 intra matmul can use N=256
    m0 = np.zeros((128, 512), dtype=np.float32)
    m0[:, 0:256] = (np.arange(256)[None, :] >= np.arange(128)[:, None])
    m0[:, 384:512] = (np.arange(128)[None, :] >= np.arange(128)[:, None])
    zm = np.zeros((128, 32), dtype=np.float32)
    for p in range(4):
        zm[0:64, p * 8 + 2 * p] = 1.0
        zm[64:128, p * 8 + 2 * p + 1] = 1.0
    zb = np.zeros((8, 512), dtype=np.float32)
    for p in range(4):
        zb[2 * p, p * 128: p * 128 + 64] = 1.0
        zb[2 * p + 1, p * 128 + 64: p * 128 + 128] = 1.0

    xTs = [np.ascontiguousarray(x[b].T) for b in range(B)]
    ins = []
    for c in range(8):
        b, hg = divmod(c, 2)
        r0 = hg * 512
        wt_c = np.ascontiguousarray(
            np.concatenate(
                [w[r0:r0 + 512], w[1024 + r0:1024 + r0 + 512],
                 w[2048 + r0:2048 + r0 + 512]], axis=0).T)     # (1024, 1536)
        wo_c = np.ascontiguousarray(
            w_out[r0:r0 + 512].reshape(8, 64, D))              # per-head rows
        ins.append({"xt": xTs[b], "wt": wt_c, "wo": wo_c,
                    "m0": m0, "zm": zm, "zb": zb})
    return ins


def kernel(x, w, w_out):
    nc = _get_nc()
    ins = _host_inputs(x, w, w_out)
    res = None
    last_err = None
    for backoff in (0.0, 5.0, 20.0, 45.0):  # axon devices fault transiently
        if backoff:
            import time as _time
            _time.sleep(backoff)
        try:
            res = run_bass_kernel_spmd(nc, ins, core_ids=list(range(8)))
            break
        except Exception as e:   # noqa: BLE001
            last_err = e
    if res is None:
        raise last_err
    out = np.empty((B, T, D), dtype=np.float32)
    for b in range(B):
        out[b] = res.results[2 * b]["out"] + res.results[2 * b + 1]["out"]
    return out
